# revision 8
# baseline (speedup 1.0000x reference)
"""GCN layer (improved self-loops) on 8 Trainium2 NeuronCores — v2.

out = D^{-1/2} (A + 2I) D^{-1/2} X W + b,  deg = in_count + 2.

Key HW facts driving this design (measured on TRN2):
  - SWDGE dma_gather descriptor generation on the GpSimd Q7 costs ~7.7ns
    per *index* regardless of element size -> gather 512B pairs (2 node
    rows per descriptor) instead of 256B singles to halve descgen time.
  - SWDGE descgen holds the shared SBUF port pair for the whole
    instruction, fully blocking concurrent DVE perf-mode ops -> do NOT
    build one-hot scatter matrices on the vector engine; stage them
    pre-built from the host and stream via HWDGE DMA (SDMA engines are
    ~15% busy, bandwidth is free).
  - PE cost per 128-col matmul is ~456ns (LDW+MM) -> align buckets to
    128 pair-slots so every 128-slot chunk maps to exactly one dst tile
    (no straddle duplication).

Structure (SPMD, one program for all 8 cores; only input data differs):
  - Nodes sharded by destination: core m owns rows [m*12544, (m+1)*12544).
  - Edges bucketed per (dst tile, src chunk); bucket edges are paired;
    each pair becomes one 512B gather descriptor reading two adjacent
    rows of a per-core Euler-walk ordered table (duplication ~6%).
  - Aggregation: for each 128-pair chunk q (owned by one dst tile):
      psum[f, d] += M2[:, q, 0:128]^T @ S_A   (A-side edges)
      psum[f, d] += M2[:, q, 128:256]^T @ S_B (B-side edges)
    where S_A/S_B are host-staged [128, 128] fp16 matrices carrying the
    full gcn_norm weight n2 = dinv[src]*dinv[dst] at [pair_slot, dst&127].
  - Self loops: per-tile staged diag(2*dinv^2) matrices against a plain
    strided load of the core's own x rows.
  - Then out = W^T agg per psum pair; bias is added on the host.
"""

import sys

sys.path.insert(0, "/opt/trn_rl_repo")

import numpy as np

import bass_rust
import concourse.bacc as bacc
import concourse.mybir as mybir
import concourse.tile as tile

F16 = mybir.dt.float16
F32 = mybir.dt.float32
I16 = mybir.dt.int16

N = 100000
FEAT = 128
NCORES = 8
PC = 12544            # nodes per core
NPAD = PC * NCORES    # 100352
TILES = PC // 128     # 98
NCHUNK = 4
CHUNK = NPAD // NCHUNK  # 25088 source rows per chunk sub-table
WAVE_SIZES = [16, 16, 16, 16, 16, 16, 2]  # tiles per wave; tiny last wave
                                          # so almost no compute is exposed
                                          # after the final descgen call


def _build_walks(pairs_full, singles):
    """Euler-trail decomposition of the pair multigraph of one (core, chunk).

    pairs_full: list of (sA, sB, pid) — full pairs (local src ids).
    singles:    list of (sA, pid) — odd-bucket leftovers (B side is padding).
    Returns (rows, idx_of_pid) where rows is the sub-table row order and
    idx_of_pid[pid] = (table position, flipped) — flipped means the walk
    traversed the pair sB->sA so the A-side row is sB.

    Method: per connected component, pair up odd-degree vertices with
    virtual edges, build an Euler circuit (Hierholzer, valid since all
    degrees even), then split the circuit at the virtual edges.
    """
    nreal = len(pairs_full)
    adj = {}  # vertex -> list of (edge_id, other)
    edges = []  # (a, b) incl. virtual

    def add_edge(a, b):
        eid = len(edges)
        edges.append((a, b))
        adj.setdefault(a, []).append((eid, b))
        adj.setdefault(b, []).append((eid, a))
        return eid

    for (a, b, _pid) in pairs_full:
        add_edge(a, b)

    # connected components + odd vertices per component
    comp = {}
    comps = []
    for v0 in adj:
        if v0 in comp:
            continue
        cid = len(comps)
        stack = [v0]
        comp[v0] = cid
        verts = [v0]
        while stack:
            v = stack.pop()
            for (_e, w) in adj[v]:
                if w not in comp:
                    comp[w] = cid
                    verts.append(w)
                    stack.append(w)
        comps.append(verts)

    for verts in comps:
        odd = [v for v in verts if len(adj[v]) & 1]
        for i in range(0, len(odd), 2):
            add_edge(odd[i], odd[i + 1])

    used = np.zeros(len(edges), dtype=bool)
    ptr = {v: 0 for v in adj}
    rows = []
    idx_of_pid = {}

    for verts in comps:
        v0 = verts[0]
        # Hierholzer Euler circuit from v0 (all degrees even now)
        stack = [(v0, -1)]
        trail_v = []
        trail_e = []
        while stack:
            v, _ = stack[-1]
            al = adj[v]
            p = ptr[v]
            while p < len(al) and used[al[p][0]]:
                p += 1
            ptr[v] = p
            if p < len(al):
                eid, w = al[p]
                used[eid] = True
                stack.append((w, eid))
            else:
                vv, ee = stack.pop()
                trail_v.append(vv)
                if ee >= 0:
                    trail_e.append(ee)
        trail_v.reverse()
        trail_e.reverse()
        L = len(trail_e)
        if L == 0:
            continue
        # rotate so a virtual edge (if any) is last
        virt_pos = [i for i, e in enumerate(trail_e) if e >= nreal]
        if virt_pos:
            p = virt_pos[0]
            trail_e = trail_e[p + 1 :] + trail_e[: p + 1]
            trail_v = trail_v[p + 1 : L + 1] + trail_v[1 : p + 2]
        # emit segments split at virtual edges
        seg_start = 0  # index into trail_e
        i = 0
        while i <= L:
            if i == L or trail_e[i] >= nreal:
                if i > seg_start:
                    base = len(rows)
                    rows.extend(trail_v[seg_start : i + 1])
                    for k in range(seg_start, i):
                        eid = trail_e[k]
                        a, b, pid = pairs_full[eid]
                        flipped = trail_v[k] != a
                        idx_of_pid[pid] = (base + (k - seg_start), flipped)
                seg_start = i + 1
            i += 1

    # singles: need any table position whose row == sA
    pos_of = {}
    for i, r in enumerate(rows):
        if r not in pos_of:
            pos_of[r] = i
    for (a, pid) in singles:
        if a in pos_of:
            idx_of_pid[pid] = (pos_of[a], False)
        else:
            pos_of[a] = len(rows)
            idx_of_pid[pid] = (len(rows), False)
            rows.append(a)
    return rows, idx_of_pid


def build_plan(src, dst, cnt):
    """Host-side metadata. src/dst int64 [E]; cnt int64 [N] in-degree."""
    E = src.shape[0]
    core = dst // PC
    tl = (dst % PC) >> 7          # dst tile within core
    ch = src // CHUNK             # source chunk

    dinv = np.zeros(NPAD, np.float64)
    dinv[:N] = 1.0 / np.sqrt(cnt + 2.0)

    flat = (core * TILES + tl) * NCHUNK + ch
    bc = np.bincount(flat, minlength=NCORES * TILES * NCHUNK).reshape(
        NCORES, TILES, NCHUNK)
    pairs_c = -(-bc // 2)                        # ceil(bucket/2) per core
    cap = pairs_c.max(axis=0)                    # [TILES, NCHUNK] max pairs
    cap = np.maximum(-(-cap // 128) * 128, 128)  # pair-slot capacity, mult 128

    assert sum(WAVE_SIZES) == TILES
    waves = []
    t0 = 0
    for ws in WAVE_SIZES:
        waves.append(list(range(t0, t0 + ws)))
        t0 += ws

    # ---- static chunk layout (shared across cores) ----
    # order: wave g -> chunk c -> tile t (in wave) -> bucket 128-block
    # chunk ordinal q; pair-slot s in [128q, 128q+128)
    bucket_base = np.zeros((TILES, NCHUNK), np.int64)  # first pair-slot
    call_nidx = []    # [g][c] -> num pair-slots (mult of 128)
    call_g16 = []     # [g][c] -> eidx col16 base
    tile_chunks = [[] for _ in range(TILES)]  # t -> [(c, q, j_local)]
    q = 0
    g16 = 0
    pos = 0
    for g, wave in enumerate(waves):
        nidx_w, g16_w = [], []
        for c in range(NCHUNK):
            nidx = int(sum(cap[t, c] for t in wave))
            nidx_w.append(nidx)
            g16_w.append(g16)
            j = 0
            for t in wave:
                bucket_base[t, c] = pos
                for _blk in range(cap[t, c] // 128):
                    tile_chunks[t].append((c, q, j))
                    q += 1
                    j += 1
                    pos += 128
            g16 += nidx // 16
        call_nidx.append(nidx_w)
        call_g16.append(g16_w)
    total_pairs = pos
    nq = q
    gcols16 = g16

    # ---- per-core data ----
    # assign each edge to (pair slot, side)
    order = np.argsort(flat * np.int64(1), kind="stable")
    flat_s = flat[order]
    starts = np.searchsorted(flat_s, np.arange(NCORES * TILES * NCHUNK))
    rank = np.arange(E) - starts[flat_s]
    bb = np.broadcast_to(bucket_base, (NCORES, TILES, NCHUNK)).reshape(-1)
    slot_s = bb[flat_s] + (rank >> 1)
    side_s = rank & 1
    src_s = src[order]
    dst_s = dst[order]
    core_s = core[order]
    ch_s = ch[order]

    eidx = np.zeros((NCORES, 128, gcols16), np.int16)
    s_host = np.zeros((NCORES, 128, nq * 256), np.float16)
    subcaps = []

    for m in range(NCORES):
        sel = core_s == m
        m_slot = slot_s[sel]
        m_side = side_s[sel]
        m_src = src_s[sel]
        m_dst = dst_s[sel]
        m_ch = ch_s[sel]
        idx_val = np.zeros(total_pairs, np.int64)  # per pair slot
        # A/B edge arrays per slot
        a_src = np.full(total_pairs, -1, np.int64)
        b_src = np.full(total_pairs, -1, np.int64)
        a_dst = np.zeros(total_pairs, np.int64)
        b_dst = np.zeros(total_pairs, np.int64)
        a_src[m_slot[m_side == 0]] = m_src[m_side == 0]
        a_dst[m_slot[m_side == 0]] = m_dst[m_side == 0]
        b_src[m_slot[m_side == 1]] = m_src[m_side == 1]
        b_dst[m_slot[m_side == 1]] = m_dst[m_side == 1]

        core_tabs = []
        for c in range(NCHUNK):
            # slots of this chunk across all waves
            csl = np.concatenate([
                np.arange(bucket_base[t, c], bucket_base[t, c] + cap[t, c])
                for t in range(TILES)])
            full = csl[(a_src[csl] >= 0) & (b_src[csl] >= 0)]
            single = csl[(a_src[csl] >= 0) & (b_src[csl] < 0)]
            pairs_full = [(int(a_src[s] - c * CHUNK), int(b_src[s] - c * CHUNK), int(s))
                          for s in full]
            singles = [(int(a_src[s] - c * CHUNK), int(s)) for s in single]
            rows, idx_of = _build_walks(pairs_full, singles)
            assert len(rows) + 1 < 32768, len(rows)
            core_tabs.append(rows)
            for s in full:
                p, fl = idx_of[int(s)]
                idx_val[s] = p
                if fl:
                    a_src[s], b_src[s] = b_src[s], a_src[s]
                    a_dst[s], b_dst[s] = b_dst[s], a_dst[s]
            for s in single:
                p, _ = idx_of[int(s)]
                idx_val[s] = p
        subcaps.append(core_tabs)

        # eidx wrap-16, replicate 8x across partitions
        w16 = idx_val.reshape(-1, 16).T.astype(np.int16)
        eidx[m] = np.tile(w16, (8, 1))

        # S matrices: [128 part=slot&127, col = q*256 + side*128 + (dst&127)]
        sl = np.arange(total_pairs)
        qq = sl >> 7
        pr = sl & 127
        av = a_src >= 0
        n2a = dinv[np.minimum(a_src, NPAD - 1)] * dinv[a_dst] * av
        bv = b_src >= 0
        n2b = dinv[np.minimum(b_src, NPAD - 1)] * dinv[b_dst] * bv
        s_host[m][pr[av], qq[av] * 256 + (a_dst[av] & 127)] = n2a[av].astype(np.float16)
        s_host[m][pr[bv], qq[bv] * 256 + 128 + (b_dst[bv] & 127)] = n2b[bv].astype(np.float16)

    SUBCAP = max(len(t) + 1 for m in range(NCORES) for t in subcaps[m])
    SUBCAP = int(-(-SUBCAP // 16) * 16)
    assert SUBCAP <= 32767

    # self-loop diag matrices [128, TILES*128]
    s_self = np.zeros((NCORES, 128, TILES * 128), np.float16)
    jj = np.arange(PC)
    for m in range(NCORES):
        g = m * PC + jj
        val = (2.0 * dinv[g] * dinv[g]).astype(np.float16)
        s_self[m][jj & 127, (jj >> 7) * 128 + (jj & 127)] = val

    return dict(
        cap=cap, waves=waves, call_nidx=call_nidx, call_g16=call_g16,
        tile_chunks=tile_chunks, total_pairs=total_pairs, nq=nq,
        gcols16=gcols16, SUBCAP=SUBCAP, subcaps=subcaps,
        eidx=eidx, s_host=s_host, s_self=s_self,
        wave_q0=[min(tile_chunks[t][0][1] for t in wave) for wave in waves],
    )


def build_bass(plan):
    waves = plan["waves"]
    call_nidx = plan["call_nidx"]
    gcols16 = plan["gcols16"]
    SUBCAP = plan["SUBCAP"]
    nq = plan["nq"]

    nc = bacc.Bacc("TRN2", target_bir_lowering=False, debug=False)
    xt = nc.dram_tensor("xt", [NCHUNK * SUBCAP + 1, FEAT], F16, kind="ExternalInput")
    xself = nc.dram_tensor("xself", [PC, FEAT], F16, kind="ExternalInput")
    eidx_d = nc.dram_tensor("eidx", [128, gcols16], I16, kind="ExternalInput")
    s_d = nc.dram_tensor("smat", [128, nq * 256], F16, kind="ExternalInput")
    sself_d = nc.dram_tensor("sself", [128, TILES * 128], F16, kind="ExternalInput")
    w_d = nc.dram_tensor("w", [FEAT, FEAT], F16, kind="ExternalInput")
    outT = nc.dram_tensor("outT", [FEAT, PC], F16, kind="ExternalOutput")

    with tile.TileContext(nc) as tc:
        with (
            tc.tile_pool(name="meta", bufs=1) as meta,
            tc.tile_pool(name="mg", bufs=2) as mgp,
            tc.tile_pool(name="sw", bufs=2) as swp,
            tc.tile_pool(name="ms", bufs=2) as msp,
            tc.tile_pool(name="fin", bufs=4) as fin,
            tc.tile_pool(name="aggps", bufs=6, space="PSUM") as aggps,
            tc.tile_pool(name="outps", bufs=2, space="PSUM") as outps,
        ):
            sb_w = meta.tile([FEAT, FEAT], F16, tag="w")
            nc.sync.dma_start(sb_w[:], w_d[:])

            for g, wave in enumerate(waves):
                nsw = len(wave)
                # per-wave gather-index slice (small, so the first descgen
                # call starts as early as possible)
                eg0 = plan["call_g16"][g][0]
                eg1 = plan["call_g16"][g][NCHUNK - 1] + call_nidx[g][NCHUNK - 1] // 16
                sb_eidx = mgp.tile([128, eg1 - eg0], I16, tag="eidx")
                nc.sync.dma_start(sb_eidx[:], eidx_d[:, eg0:eg1])
                # gathers (SWDGE): 4 calls, 512B pair descriptors
                mtiles = {}
                for c in range(NCHUNK):
                    nidx = call_nidx[g][c]
                    if nidx == 0:
                        continue
                    m2 = mgp.tile([128, nidx // 128, 256], F16, tag=f"mg{c}")
                    g16 = plan["call_g16"][g][c]
                    in_ap = xt[:, :].copy()
                    in_ap.ap = bass_rust.VecI64Pair([(FEAT, SUBCAP), (1, 256)])
                    in_ap.offset = c * SUBCAP * FEAT
                    nc.gpsimd.dma_gather(
                        m2[:, : nidx // 128, :],
                        in_ap,
                        sb_eidx[:, g16 - eg0 : g16 - eg0 + nidx // 16],
                        nidx, nidx, 256,
                        elem_step=FEAT,
                        single_packet=(nidx <= 1024),
                    )
                    mtiles[c] = m2

                # HWDGE loads: S block, self rows, self S
                q0 = plan["wave_q0"][g]
                nqw = sum(call_nidx[g]) // 128
                sw = swp.tile([128, nqw * 256], F16, tag="sw")
                nc.sync.dma_start(sw[:], s_d[:, q0 * 256 : (q0 + nqw) * 256])
                ms = msp.tile([128, nsw, 128], F16, tag="ms")
                r0 = wave[0] * 128
                nc.sync.dma_start(
                    ms[:], xself[r0 : r0 + nsw * 128, :].rearrange(
                        "(n p) d -> p n d", p=128))
                ssl = msp.tile([128, nsw * 128], F16, tag="ssl")
                nc.sync.dma_start(
                    ssl[:], sself_d[:, wave[0] * 128 : (wave[0] + nsw) * 128])

                for p0 in range(0, nsw, 2):
                    t0, t1 = wave[p0], wave[p0 + 1]
                    ppair = aggps.tile([128, 256], F32, tag="agg", name="agg")
                    for half, t in ((0, t0), (128, t1)):
                        first = True
                        for (c, qq, j) in plan["tile_chunks"][t]:
                            lq = qq - q0
                            for side in (0, 1):
                                nc.tensor.matmul(
                                    ppair[:, half : half + 128],
                                    mtiles[c][:, j, side * 128 : side * 128 + 128],
                                    sw[:, lq * 256 + side * 128 : lq * 256 + side * 128 + 128],
                                    start=first, stop=False,
                                    skip_group_check=True,
                                )
                                first = False
                        tig = t - wave[0]
                        nc.tensor.matmul(
                            ppair[:, half : half + 128],
                            ms[:, tig, :],
                            ssl[:, tig * 128 : tig * 128 + 128],
                            start=first, stop=True, skip_group_check=True,
                        )
                    asb = fin.tile([128, 256], F16, tag="asb")
                    nc.scalar.activation(
                        asb[:], ppair[:], mybir.ActivationFunctionType.Identity)
                    op = outps.tile([128, 256], F32, tag="op")
                    nc.tensor.matmul(op[:], sb_w[:], asb[:], skip_group_check=True)
                    osb = fin.tile([128, 256], F16, tag="osb")
                    nc.scalar.activation(
                        osb[:], op[:], mybir.ActivationFunctionType.Identity)
                    nc.sync.dma_start(outT[:, t0 * 128 : t0 * 128 + 256], osb[:])
    nc.compile()
    return nc


_CACHE = {}


def _get_compiled(src, dst, cnt):
    plan = build_plan(src, dst, cnt)
    key = (plan["SUBCAP"], plan["cap"].tobytes())
    if key not in _CACHE:
        _CACHE[key] = (build_bass(plan), plan)
    else:
        _CACHE[key] = (_CACHE[key][0], plan)
    return _CACHE[key]


def make_inputs(plan, x, W):
    """Per-core input maps (everything except the run itself)."""
    xf = np.asarray(x).astype(np.float16)
    SUBCAP = plan["SUBCAP"]
    in_maps = []
    for m in range(NCORES):
        xtab = np.zeros((NCHUNK * SUBCAP + 1, FEAT), np.float16)
        for c in range(NCHUNK):
            rows = np.asarray(plan["subcaps"][m][c], np.int64)
            if rows.size:
                xtab[c * SUBCAP : c * SUBCAP + rows.size] = xf[
                    np.minimum(rows + c * CHUNK, N - 1)] * (rows + c * CHUNK < N)[:, None].astype(np.float16)
        xs = np.zeros((PC, FEAT), np.float16)
        lo = m * PC
        hi = min((m + 1) * PC, N)
        xs[: hi - lo] = xf[lo:hi]
        in_maps.append({
            "xt": xtab,
            "xself": xs,
            "eidx": plan["eidx"][m],
            "smat": plan["s_host"][m],
            "sself": plan["s_self"][m],
            "w": np.asarray(W).astype(np.float16),
        })
    return in_maps


def kernel(x, edge_index, W, b):
    from concourse.bass_utils import run_bass_kernel_spmd

    x = np.asarray(x)
    edge_index = np.asarray(edge_index)
    W = np.asarray(W)
    b = np.asarray(b)
    src = edge_index[0].astype(np.int64)
    dst = edge_index[1].astype(np.int64)
    cnt = np.bincount(dst, minlength=N)

    nc, plan = _get_compiled(src, dst, cnt)
    in_maps = make_inputs(plan, x, W)
    res = run_bass_kernel_spmd(nc, in_maps, list(range(NCORES)))
    outT = np.concatenate([res.results[m]["outT"] for m in range(NCORES)], axis=1)
    return (outT[:, :N].T.astype(np.float32) + b.astype(np.float32)[None, :])


# revision 10
# speedup vs baseline: 1.1601x; 1.1601x over previous
"""GCN layer (improved self-loops) on 8 Trainium2 NeuronCores — v2.

out = D^{-1/2} (A + 2I) D^{-1/2} X W + b,  deg = in_count + 2.

Key HW facts driving this design (measured on TRN2):
  - SWDGE dma_gather descriptor generation on the GpSimd Q7 costs ~7.7ns
    per *index* regardless of element size -> gather 512B pairs (2 node
    rows per descriptor) instead of 256B singles to halve descgen time.
  - SWDGE descgen holds the shared SBUF port pair for the whole
    instruction, fully blocking concurrent DVE perf-mode ops -> do NOT
    build one-hot scatter matrices on the vector engine; stage them
    pre-built from the host and stream via HWDGE DMA (SDMA engines are
    ~15% busy, bandwidth is free).
  - PE cost per 128-col matmul is ~456ns (LDW+MM) -> align buckets to
    128 pair-slots so every 128-slot chunk maps to exactly one dst tile
    (no straddle duplication).

Structure (SPMD, one program for all 8 cores; only input data differs):
  - Nodes sharded by destination: core m owns rows [m*12544, (m+1)*12544).
  - Edges bucketed per (dst tile, src chunk); bucket edges are paired;
    each pair becomes one 512B gather descriptor reading two adjacent
    rows of a per-core Euler-walk ordered table (duplication ~6%).
  - Aggregation: for each 128-pair chunk q (owned by one dst tile):
      psum[f, d] += M2[:, q, 0:128]^T @ S_A   (A-side edges)
      psum[f, d] += M2[:, q, 128:256]^T @ S_B (B-side edges)
    where S_A/S_B are host-staged [128, 128] fp16 matrices carrying the
    full gcn_norm weight n2 = dinv[src]*dinv[dst] at [pair_slot, dst&127].
  - Self loops: per-tile staged diag(2*dinv^2) matrices against a plain
    strided load of the core's own x rows.
  - Then out = W^T agg per psum pair; bias is added on the host.
"""

import sys

sys.path.insert(0, "/opt/trn_rl_repo")

import numpy as np

import bass_rust
import concourse.bacc as bacc
import concourse.mybir as mybir
import concourse.tile as tile

F16 = mybir.dt.float16
F32 = mybir.dt.float32
I16 = mybir.dt.int16

N = 100000
FEAT = 128
NCORES = 8
PC = 12544            # nodes per core
NPAD = PC * NCORES    # 100352
TILES = PC // 128     # 98
NCHUNK = 4
CHUNK = NPAD // NCHUNK  # 25088 source rows per chunk sub-table
WAVE_SIZES = [16, 16, 16, 16, 16, 16, 2]  # tiles per wave; tiny last wave
                                          # so almost no compute is exposed
                                          # after the final descgen call


def _build_walks(pairs_full, singles):
    """Euler-trail decomposition of the pair multigraph of one (core, chunk).

    pairs_full: list of (sA, sB, pid) — full pairs (local src ids).
    singles:    list of (sA, pid) — odd-bucket leftovers (B side is padding).
    Returns (rows, idx_of_pid) where rows is the sub-table row order and
    idx_of_pid[pid] = (table position, flipped) — flipped means the walk
    traversed the pair sB->sA so the A-side row is sB.

    Method: per connected component, pair up odd-degree vertices with
    virtual edges, build an Euler circuit (Hierholzer, valid since all
    degrees even), then split the circuit at the virtual edges.
    """
    nreal = len(pairs_full)
    adj = {}  # vertex -> list of (edge_id, other)
    edges = []  # (a, b) incl. virtual

    def add_edge(a, b):
        eid = len(edges)
        edges.append((a, b))
        adj.setdefault(a, []).append((eid, b))
        adj.setdefault(b, []).append((eid, a))
        return eid

    for (a, b, _pid) in pairs_full:
        add_edge(a, b)

    # connected components + odd vertices per component
    comp = {}
    comps = []
    for v0 in adj:
        if v0 in comp:
            continue
        cid = len(comps)
        stack = [v0]
        comp[v0] = cid
        verts = [v0]
        while stack:
            v = stack.pop()
            for (_e, w) in adj[v]:
                if w not in comp:
                    comp[w] = cid
                    verts.append(w)
                    stack.append(w)
        comps.append(verts)

    for verts in comps:
        odd = [v for v in verts if len(adj[v]) & 1]
        for i in range(0, len(odd), 2):
            add_edge(odd[i], odd[i + 1])

    used = np.zeros(len(edges), dtype=bool)
    ptr = {v: 0 for v in adj}
    rows = []
    idx_of_pid = {}

    for verts in comps:
        v0 = verts[0]
        # Hierholzer Euler circuit from v0 (all degrees even now)
        stack = [(v0, -1)]
        trail_v = []
        trail_e = []
        while stack:
            v, _ = stack[-1]
            al = adj[v]
            p = ptr[v]
            while p < len(al) and used[al[p][0]]:
                p += 1
            ptr[v] = p
            if p < len(al):
                eid, w = al[p]
                used[eid] = True
                stack.append((w, eid))
            else:
                vv, ee = stack.pop()
                trail_v.append(vv)
                if ee >= 0:
                    trail_e.append(ee)
        trail_v.reverse()
        trail_e.reverse()
        L = len(trail_e)
        if L == 0:
            continue
        # rotate so a virtual edge (if any) is last
        virt_pos = [i for i, e in enumerate(trail_e) if e >= nreal]
        if virt_pos:
            p = virt_pos[0]
            trail_e = trail_e[p + 1 :] + trail_e[: p + 1]
            trail_v = trail_v[p + 1 : L + 1] + trail_v[1 : p + 2]
        # emit segments split at virtual edges
        seg_start = 0  # index into trail_e
        i = 0
        while i <= L:
            if i == L or trail_e[i] >= nreal:
                if i > seg_start:
                    base = len(rows)
                    rows.extend(trail_v[seg_start : i + 1])
                    for k in range(seg_start, i):
                        eid = trail_e[k]
                        a, b, pid = pairs_full[eid]
                        flipped = trail_v[k] != a
                        idx_of_pid[pid] = (base + (k - seg_start), flipped)
                seg_start = i + 1
            i += 1

    # singles: need any table position whose row == sA
    pos_of = {}
    for i, r in enumerate(rows):
        if r not in pos_of:
            pos_of[r] = i
    for (a, pid) in singles:
        if a in pos_of:
            idx_of_pid[pid] = (pos_of[a], False)
        else:
            pos_of[a] = len(rows)
            idx_of_pid[pid] = (len(rows), False)
            rows.append(a)
    return rows, idx_of_pid


def build_plan(src, dst, cnt):
    """Host-side metadata. src/dst int64 [E]; cnt int64 [N] in-degree."""
    E = src.shape[0]
    core = dst // PC
    tl = (dst % PC) >> 7          # dst tile within core
    ch = src // CHUNK             # source chunk

    dinv = np.zeros(NPAD, np.float64)
    dinv[:N] = 1.0 / np.sqrt(cnt + 2.0)

    flat = (core * TILES + tl) * NCHUNK + ch
    bc = np.bincount(flat, minlength=NCORES * TILES * NCHUNK).reshape(
        NCORES, TILES, NCHUNK)
    pairs_c = -(-bc // 2)                        # ceil(bucket/2) per core
    cap = pairs_c.max(axis=0)                    # [TILES, NCHUNK] max pairs
    cap = np.maximum(-(-cap // 128) * 128, 128)  # pair-slot capacity, mult 128

    assert sum(WAVE_SIZES) == TILES
    waves = []
    t0 = 0
    for ws in WAVE_SIZES:
        waves.append(list(range(t0, t0 + ws)))
        t0 += ws

    # ---- static chunk layout (shared across cores) ----
    # order: wave g -> chunk c -> tile t (in wave) -> bucket 128-block
    # chunk ordinal q; pair-slot s in [128q, 128q+128)
    bucket_base = np.zeros((TILES, NCHUNK), np.int64)  # first pair-slot
    call_nidx = []    # [g][c] -> num pair-slots (mult of 128)
    call_g16 = []     # [g][c] -> eidx col16 base
    tile_chunks = [[] for _ in range(TILES)]  # t -> [(c, q, j_local)]
    q = 0
    g16 = 0
    pos = 0
    for g, wave in enumerate(waves):
        nidx_w, g16_w = [], []
        for c in range(NCHUNK):
            nidx = int(sum(cap[t, c] for t in wave))
            nidx_w.append(nidx)
            g16_w.append(g16)
            j = 0
            for t in wave:
                bucket_base[t, c] = pos
                for _blk in range(cap[t, c] // 128):
                    tile_chunks[t].append((c, q, j))
                    q += 1
                    j += 1
                    pos += 128
            g16 += nidx // 16
        call_nidx.append(nidx_w)
        call_g16.append(g16_w)
    total_pairs = pos
    nq = q
    gcols16 = g16

    # ---- per-core data ----
    # assign each edge to (pair slot, side)
    order = np.argsort(flat * np.int64(1), kind="stable")
    flat_s = flat[order]
    starts = np.searchsorted(flat_s, np.arange(NCORES * TILES * NCHUNK))
    rank = np.arange(E) - starts[flat_s]
    bb = np.broadcast_to(bucket_base, (NCORES, TILES, NCHUNK)).reshape(-1)
    slot_s = bb[flat_s] + (rank >> 1)
    side_s = rank & 1
    src_s = src[order]
    dst_s = dst[order]
    core_s = core[order]
    ch_s = ch[order]

    eidx = np.zeros((NCORES, 128, gcols16), np.int16)
    s_host = np.zeros((NCORES, 128, nq * 256), np.float16)
    subcaps = []

    for m in range(NCORES):
        sel = core_s == m
        m_slot = slot_s[sel]
        m_side = side_s[sel]
        m_src = src_s[sel]
        m_dst = dst_s[sel]
        m_ch = ch_s[sel]
        idx_val = np.zeros(total_pairs, np.int64)  # per pair slot
        # A/B edge arrays per slot
        a_src = np.full(total_pairs, -1, np.int64)
        b_src = np.full(total_pairs, -1, np.int64)
        a_dst = np.zeros(total_pairs, np.int64)
        b_dst = np.zeros(total_pairs, np.int64)
        a_src[m_slot[m_side == 0]] = m_src[m_side == 0]
        a_dst[m_slot[m_side == 0]] = m_dst[m_side == 0]
        b_src[m_slot[m_side == 1]] = m_src[m_side == 1]
        b_dst[m_slot[m_side == 1]] = m_dst[m_side == 1]

        core_tabs = []
        for c in range(NCHUNK):
            # slots of this chunk across all waves
            csl = np.concatenate([
                np.arange(bucket_base[t, c], bucket_base[t, c] + cap[t, c])
                for t in range(TILES)])
            full = csl[(a_src[csl] >= 0) & (b_src[csl] >= 0)]
            single = csl[(a_src[csl] >= 0) & (b_src[csl] < 0)]
            pairs_full = [(int(a_src[s] - c * CHUNK), int(b_src[s] - c * CHUNK), int(s))
                          for s in full]
            singles = [(int(a_src[s] - c * CHUNK), int(s)) for s in single]
            rows, idx_of = _build_walks(pairs_full, singles)
            assert len(rows) + 1 < 32768, len(rows)
            core_tabs.append(rows)
            for s in full:
                p, fl = idx_of[int(s)]
                idx_val[s] = p
                if fl:
                    a_src[s], b_src[s] = b_src[s], a_src[s]
                    a_dst[s], b_dst[s] = b_dst[s], a_dst[s]
            for s in single:
                p, _ = idx_of[int(s)]
                idx_val[s] = p
        subcaps.append(core_tabs)

        # eidx wrap-16, replicate 8x across partitions
        w16 = idx_val.reshape(-1, 16).T.astype(np.int16)
        eidx[m] = np.tile(w16, (8, 1))

        # S matrices: [128 part=slot&127, col = q*256 + side*128 + (dst&127)]
        sl = np.arange(total_pairs)
        qq = sl >> 7
        pr = sl & 127
        av = a_src >= 0
        n2a = dinv[np.minimum(a_src, NPAD - 1)] * dinv[a_dst] * av
        bv = b_src >= 0
        n2b = dinv[np.minimum(b_src, NPAD - 1)] * dinv[b_dst] * bv
        s_host[m][pr[av], qq[av] * 256 + (a_dst[av] & 127)] = n2a[av].astype(np.float16)
        s_host[m][pr[bv], qq[bv] * 256 + 128 + (b_dst[bv] & 127)] = n2b[bv].astype(np.float16)

    SUBCAP = max(len(t) + 1 for m in range(NCORES) for t in subcaps[m])
    SUBCAP = int(-(-SUBCAP // 16) * 16)
    assert SUBCAP <= 32767

    # self-loop diag matrices [128, TILES*128]
    s_self = np.zeros((NCORES, 128, TILES * 128), np.float16)
    jj = np.arange(PC)
    for m in range(NCORES):
        g = m * PC + jj
        val = (2.0 * dinv[g] * dinv[g]).astype(np.float16)
        s_self[m][jj & 127, (jj >> 7) * 128 + (jj & 127)] = val

    return dict(
        cap=cap, waves=waves, call_nidx=call_nidx, call_g16=call_g16,
        tile_chunks=tile_chunks, total_pairs=total_pairs, nq=nq,
        gcols16=gcols16, SUBCAP=SUBCAP, subcaps=subcaps,
        eidx=eidx, s_host=s_host, s_self=s_self,
        wave_q0=[min(tile_chunks[t][0][1] for t in wave) for wave in waves],
    )


def build_bass(plan):
    waves = plan["waves"]
    call_nidx = plan["call_nidx"]
    gcols16 = plan["gcols16"]
    SUBCAP = plan["SUBCAP"]
    nq = plan["nq"]

    nc = bacc.Bacc("TRN2", target_bir_lowering=False, debug=False)
    xt = nc.dram_tensor("xt", [NCHUNK * SUBCAP + 1, FEAT], F16, kind="ExternalInput")
    xself = nc.dram_tensor("xself", [PC, FEAT], F16, kind="ExternalInput")
    eidx_d = nc.dram_tensor("eidx", [128, gcols16], I16, kind="ExternalInput")
    s_d = nc.dram_tensor("smat", [128, nq * 256], F16, kind="ExternalInput")
    sself_d = nc.dram_tensor("sself", [128, TILES * 128], F16, kind="ExternalInput")
    w_d = nc.dram_tensor("w", [FEAT, FEAT], F16, kind="ExternalInput")
    outT = nc.dram_tensor("outT", [FEAT, PC], F16, kind="ExternalOutput")

    with tile.TileContext(nc) as tc:
        with (
            tc.tile_pool(name="meta", bufs=1) as meta,
            tc.tile_pool(name="mg", bufs=2) as mgp,
            tc.tile_pool(name="sw", bufs=2) as swp,
            tc.tile_pool(name="ms", bufs=2) as msp,
            tc.tile_pool(name="fin", bufs=4) as fin,
            tc.tile_pool(name="aggps", bufs=6, space="PSUM") as aggps,
            tc.tile_pool(name="outps", bufs=2, space="PSUM") as outps,
        ):
            sb_eidx = meta.tile([128, gcols16], I16, tag="eidx")
            nc.sync.dma_start(sb_eidx[:], eidx_d[:])
            sb_w = meta.tile([FEAT, FEAT], F16, tag="w")
            nc.sync.dma_start(sb_w[:], w_d[:])

            for g, wave in enumerate(waves):
                nsw = len(wave)
                # gathers (SWDGE): 4 calls, 512B pair descriptors
                mtiles = {}
                for c in range(NCHUNK):
                    nidx = call_nidx[g][c]
                    if nidx == 0:
                        continue
                    m2 = mgp.tile([128, nidx // 128, 256], F16, tag=f"mg{c}")
                    g16 = plan["call_g16"][g][c]
                    in_ap = xt[:, :].copy()
                    in_ap.ap = bass_rust.VecI64Pair([(FEAT, SUBCAP), (1, 256)])
                    in_ap.offset = c * SUBCAP * FEAT
                    nc.gpsimd.dma_gather(
                        m2[:, : nidx // 128, :],
                        in_ap,
                        sb_eidx[:, g16 : g16 + nidx // 16],
                        nidx, nidx, 256,
                        elem_step=FEAT,
                        single_packet=(nidx <= 1024),
                    )
                    mtiles[c] = m2

                # HWDGE loads: S block, self rows, self S
                q0 = plan["wave_q0"][g]
                nqw = sum(call_nidx[g]) // 128
                sw = swp.tile([128, nqw * 256], F16, tag="sw")
                nc.sync.dma_start(sw[:], s_d[:, q0 * 256 : (q0 + nqw) * 256])
                ms = msp.tile([128, nsw, 128], F16, tag="ms")
                r0 = wave[0] * 128
                nc.sync.dma_start(
                    ms[:], xself[r0 : r0 + nsw * 128, :].rearrange(
                        "(n p) d -> p n d", p=128))
                ssl = msp.tile([128, nsw * 128], F16, tag="ssl")
                nc.sync.dma_start(
                    ssl[:], sself_d[:, wave[0] * 128 : (wave[0] + nsw) * 128])

                for p0 in range(0, nsw, 2):
                    t0, t1 = wave[p0], wave[p0 + 1]
                    ppair = aggps.tile([128, 256], F32, tag="agg", name="agg")
                    for half, t in ((0, t0), (128, t1)):
                        first = True
                        for (c, qq, j) in plan["tile_chunks"][t]:
                            lq = qq - q0
                            for side in (0, 1):
                                nc.tensor.matmul(
                                    ppair[:, half : half + 128],
                                    mtiles[c][:, j, side * 128 : side * 128 + 128],
                                    sw[:, lq * 256 + side * 128 : lq * 256 + side * 128 + 128],
                                    start=first, stop=False,
                                    skip_group_check=True,
                                )
                                first = False
                        tig = t - wave[0]
                        nc.tensor.matmul(
                            ppair[:, half : half + 128],
                            ms[:, tig, :],
                            ssl[:, tig * 128 : tig * 128 + 128],
                            start=first, stop=True, skip_group_check=True,
                        )
                    asb = fin.tile([128, 256], F16, tag="asb")
                    nc.scalar.activation(
                        asb[:], ppair[:], mybir.ActivationFunctionType.Identity)
                    op = outps.tile([128, 256], F32, tag="op")
                    nc.tensor.matmul(op[:], sb_w[:], asb[:], skip_group_check=True)
                    osb = fin.tile([128, 256], F16, tag="osb")
                    nc.scalar.activation(
                        osb[:], op[:], mybir.ActivationFunctionType.Identity)
                    # scalar-queue HWDGE so output stores never block the
                    # sync queue's next-wave loads
                    nc.scalar.dma_start(outT[:, t0 * 128 : t0 * 128 + 256], osb[:])
    nc.compile()
    return nc


_CACHE = {}


def _get_compiled(src, dst, cnt):
    plan = build_plan(src, dst, cnt)
    key = (plan["SUBCAP"], plan["cap"].tobytes())
    if key not in _CACHE:
        _CACHE[key] = (build_bass(plan), plan)
    else:
        _CACHE[key] = (_CACHE[key][0], plan)
    return _CACHE[key]


def make_inputs(plan, x, W):
    """Per-core input maps (everything except the run itself)."""
    xf = np.asarray(x).astype(np.float16)
    SUBCAP = plan["SUBCAP"]
    in_maps = []
    for m in range(NCORES):
        xtab = np.zeros((NCHUNK * SUBCAP + 1, FEAT), np.float16)
        for c in range(NCHUNK):
            rows = np.asarray(plan["subcaps"][m][c], np.int64)
            if rows.size:
                xtab[c * SUBCAP : c * SUBCAP + rows.size] = xf[
                    np.minimum(rows + c * CHUNK, N - 1)] * (rows + c * CHUNK < N)[:, None].astype(np.float16)
        xs = np.zeros((PC, FEAT), np.float16)
        lo = m * PC
        hi = min((m + 1) * PC, N)
        xs[: hi - lo] = xf[lo:hi]
        in_maps.append({
            "xt": xtab,
            "xself": xs,
            "eidx": plan["eidx"][m],
            "smat": plan["s_host"][m],
            "sself": plan["s_self"][m],
            "w": np.asarray(W).astype(np.float16),
        })
    return in_maps


def kernel(x, edge_index, W, b):
    from concourse.bass_utils import run_bass_kernel_spmd

    x = np.asarray(x)
    edge_index = np.asarray(edge_index)
    W = np.asarray(W)
    b = np.asarray(b)
    src = edge_index[0].astype(np.int64)
    dst = edge_index[1].astype(np.int64)
    cnt = np.bincount(dst, minlength=N)

    nc, plan = _get_compiled(src, dst, cnt)
    in_maps = make_inputs(plan, x, W)
    res = run_bass_kernel_spmd(nc, in_maps, list(range(NCORES)))
    outT = np.concatenate([res.results[m]["outT"] for m in range(NCORES)], axis=1)
    return (outT[:, :N].T.astype(np.float32) + b.astype(np.float32)[None, :])


# revision 11
# speedup vs baseline: 1.1898x; 1.0256x over previous
"""GCN layer (improved self-loops) on 8 Trainium2 NeuronCores — v2.

out = D^{-1/2} (A + 2I) D^{-1/2} X W + b,  deg = in_count + 2.

Key HW facts driving this design (measured on TRN2):
  - SWDGE dma_gather descriptor generation on the GpSimd Q7 costs ~7.7ns
    per *index* regardless of element size -> gather 512B pairs (2 node
    rows per descriptor) instead of 256B singles to halve descgen time.
  - SWDGE descgen holds the shared SBUF port pair for the whole
    instruction, fully blocking concurrent DVE perf-mode ops -> do NOT
    build one-hot scatter matrices on the vector engine; stage them
    pre-built from the host and stream via HWDGE DMA (SDMA engines are
    ~15% busy, bandwidth is free).
  - PE cost per 128-col matmul is ~456ns (LDW+MM) -> align buckets to
    128 pair-slots so every 128-slot chunk maps to exactly one dst tile
    (no straddle duplication).

Structure (SPMD, one program for all 8 cores; only input data differs):
  - Nodes sharded by destination: core m owns rows [m*12544, (m+1)*12544).
  - Edges bucketed per (dst tile, src chunk); bucket edges are paired;
    each pair becomes one 512B gather descriptor reading two adjacent
    rows of a per-core Euler-walk ordered table (duplication ~6%).
  - Aggregation: for each 128-pair chunk q (owned by one dst tile):
      psum[f, d] += M2[:, q, 0:128]^T @ S_A   (A-side edges)
      psum[f, d] += M2[:, q, 128:256]^T @ S_B (B-side edges)
    where S_A/S_B are host-staged [128, 128] fp16 matrices carrying the
    full gcn_norm weight n2 = dinv[src]*dinv[dst] at [pair_slot, dst&127].
  - Self loops: per-tile staged diag(2*dinv^2) matrices against a plain
    strided load of the core's own x rows.
  - Then out = W^T agg per psum pair; bias is added on the host.
"""

import sys

sys.path.insert(0, "/opt/trn_rl_repo")

import numpy as np

import bass_rust
import concourse.bacc as bacc
import concourse.mybir as mybir
import concourse.tile as tile

F16 = mybir.dt.float16
F32 = mybir.dt.float32
I16 = mybir.dt.int16

N = 100000
FEAT = 128
NCORES = 8
PC = 12544            # nodes per core
NPAD = PC * NCORES    # 100352
TILES = PC // 128     # 98
NCHUNK = 4
CHUNK = NPAD // NCHUNK  # 25088 source rows per chunk sub-table
WAVE_SIZES = [16, 16, 16, 16, 16, 10, 6, 2]  # tapered tail: wave w's compute
                                             # (~1.8us/tile) must fit under
                                             # waves w+1.. descgen (~4us/tile)


def _build_walks(pairs_full, singles):
    """Euler-trail decomposition of the pair multigraph of one (core, chunk).

    pairs_full: list of (sA, sB, pid) — full pairs (local src ids).
    singles:    list of (sA, pid) — odd-bucket leftovers (B side is padding).
    Returns (rows, idx_of_pid) where rows is the sub-table row order and
    idx_of_pid[pid] = (table position, flipped) — flipped means the walk
    traversed the pair sB->sA so the A-side row is sB.

    Method: per connected component, pair up odd-degree vertices with
    virtual edges, build an Euler circuit (Hierholzer, valid since all
    degrees even), then split the circuit at the virtual edges.
    """
    nreal = len(pairs_full)
    adj = {}  # vertex -> list of (edge_id, other)
    edges = []  # (a, b) incl. virtual

    def add_edge(a, b):
        eid = len(edges)
        edges.append((a, b))
        adj.setdefault(a, []).append((eid, b))
        adj.setdefault(b, []).append((eid, a))
        return eid

    for (a, b, _pid) in pairs_full:
        add_edge(a, b)

    # connected components + odd vertices per component
    comp = {}
    comps = []
    for v0 in adj:
        if v0 in comp:
            continue
        cid = len(comps)
        stack = [v0]
        comp[v0] = cid
        verts = [v0]
        while stack:
            v = stack.pop()
            for (_e, w) in adj[v]:
                if w not in comp:
                    comp[w] = cid
                    verts.append(w)
                    stack.append(w)
        comps.append(verts)

    for verts in comps:
        odd = [v for v in verts if len(adj[v]) & 1]
        for i in range(0, len(odd), 2):
            add_edge(odd[i], odd[i + 1])

    used = np.zeros(len(edges), dtype=bool)
    ptr = {v: 0 for v in adj}
    rows = []
    idx_of_pid = {}

    for verts in comps:
        v0 = verts[0]
        # Hierholzer Euler circuit from v0 (all degrees even now)
        stack = [(v0, -1)]
        trail_v = []
        trail_e = []
        while stack:
            v, _ = stack[-1]
            al = adj[v]
            p = ptr[v]
            while p < len(al) and used[al[p][0]]:
                p += 1
            ptr[v] = p
            if p < len(al):
                eid, w = al[p]
                used[eid] = True
                stack.append((w, eid))
            else:
                vv, ee = stack.pop()
                trail_v.append(vv)
                if ee >= 0:
                    trail_e.append(ee)
        trail_v.reverse()
        trail_e.reverse()
        L = len(trail_e)
        if L == 0:
            continue
        # rotate so a virtual edge (if any) is last
        virt_pos = [i for i, e in enumerate(trail_e) if e >= nreal]
        if virt_pos:
            p = virt_pos[0]
            trail_e = trail_e[p + 1 :] + trail_e[: p + 1]
            trail_v = trail_v[p + 1 : L + 1] + trail_v[1 : p + 2]
        # emit segments split at virtual edges
        seg_start = 0  # index into trail_e
        i = 0
        while i <= L:
            if i == L or trail_e[i] >= nreal:
                if i > seg_start:
                    base = len(rows)
                    rows.extend(trail_v[seg_start : i + 1])
                    for k in range(seg_start, i):
                        eid = trail_e[k]
                        a, b, pid = pairs_full[eid]
                        flipped = trail_v[k] != a
                        idx_of_pid[pid] = (base + (k - seg_start), flipped)
                seg_start = i + 1
            i += 1

    # singles: need any table position whose row == sA
    pos_of = {}
    for i, r in enumerate(rows):
        if r not in pos_of:
            pos_of[r] = i
    for (a, pid) in singles:
        if a in pos_of:
            idx_of_pid[pid] = (pos_of[a], False)
        else:
            pos_of[a] = len(rows)
            idx_of_pid[pid] = (len(rows), False)
            rows.append(a)
    return rows, idx_of_pid


def build_plan(src, dst, cnt):
    """Host-side metadata. src/dst int64 [E]; cnt int64 [N] in-degree."""
    E = src.shape[0]
    core = dst // PC
    tl = (dst % PC) >> 7          # dst tile within core
    ch = src // CHUNK             # source chunk

    dinv = np.zeros(NPAD, np.float64)
    dinv[:N] = 1.0 / np.sqrt(cnt + 2.0)

    flat = (core * TILES + tl) * NCHUNK + ch
    bc = np.bincount(flat, minlength=NCORES * TILES * NCHUNK).reshape(
        NCORES, TILES, NCHUNK)
    pairs_c = -(-bc // 2)                        # ceil(bucket/2) per core
    cap = pairs_c.max(axis=0)                    # [TILES, NCHUNK] max pairs
    cap = np.maximum(-(-cap // 128) * 128, 128)  # pair-slot capacity, mult 128

    assert sum(WAVE_SIZES) == TILES
    waves = []
    t0 = 0
    for ws in WAVE_SIZES:
        waves.append(list(range(t0, t0 + ws)))
        t0 += ws

    # ---- static chunk layout (shared across cores) ----
    # order: wave g -> chunk c -> tile t (in wave) -> bucket 128-block
    # chunk ordinal q; pair-slot s in [128q, 128q+128)
    bucket_base = np.zeros((TILES, NCHUNK), np.int64)  # first pair-slot
    call_nidx = []    # [g][c] -> num pair-slots (mult of 128)
    call_g16 = []     # [g][c] -> eidx col16 base
    tile_chunks = [[] for _ in range(TILES)]  # t -> [(c, q, j_local)]
    q = 0
    g16 = 0
    pos = 0
    for g, wave in enumerate(waves):
        nidx_w, g16_w = [], []
        for c in range(NCHUNK):
            nidx = int(sum(cap[t, c] for t in wave))
            nidx_w.append(nidx)
            g16_w.append(g16)
            j = 0
            for t in wave:
                bucket_base[t, c] = pos
                for _blk in range(cap[t, c] // 128):
                    tile_chunks[t].append((c, q, j))
                    q += 1
                    j += 1
                    pos += 128
            g16 += nidx // 16
        call_nidx.append(nidx_w)
        call_g16.append(g16_w)
    total_pairs = pos
    nq = q
    gcols16 = g16

    # ---- per-core data ----
    # assign each edge to (pair slot, side)
    order = np.argsort(flat * np.int64(1), kind="stable")
    flat_s = flat[order]
    starts = np.searchsorted(flat_s, np.arange(NCORES * TILES * NCHUNK))
    rank = np.arange(E) - starts[flat_s]
    bb = np.broadcast_to(bucket_base, (NCORES, TILES, NCHUNK)).reshape(-1)
    slot_s = bb[flat_s] + (rank >> 1)
    side_s = rank & 1
    src_s = src[order]
    dst_s = dst[order]
    core_s = core[order]
    ch_s = ch[order]

    eidx = np.zeros((NCORES, 128, gcols16), np.int16)
    s_host = np.zeros((NCORES, 128, nq * 256), np.float16)
    subcaps = []

    for m in range(NCORES):
        sel = core_s == m
        m_slot = slot_s[sel]
        m_side = side_s[sel]
        m_src = src_s[sel]
        m_dst = dst_s[sel]
        m_ch = ch_s[sel]
        idx_val = np.zeros(total_pairs, np.int64)  # per pair slot
        # A/B edge arrays per slot
        a_src = np.full(total_pairs, -1, np.int64)
        b_src = np.full(total_pairs, -1, np.int64)
        a_dst = np.zeros(total_pairs, np.int64)
        b_dst = np.zeros(total_pairs, np.int64)
        a_src[m_slot[m_side == 0]] = m_src[m_side == 0]
        a_dst[m_slot[m_side == 0]] = m_dst[m_side == 0]
        b_src[m_slot[m_side == 1]] = m_src[m_side == 1]
        b_dst[m_slot[m_side == 1]] = m_dst[m_side == 1]

        core_tabs = []
        for c in range(NCHUNK):
            # slots of this chunk across all waves
            csl = np.concatenate([
                np.arange(bucket_base[t, c], bucket_base[t, c] + cap[t, c])
                for t in range(TILES)])
            full = csl[(a_src[csl] >= 0) & (b_src[csl] >= 0)]
            single = csl[(a_src[csl] >= 0) & (b_src[csl] < 0)]
            pairs_full = [(int(a_src[s] - c * CHUNK), int(b_src[s] - c * CHUNK), int(s))
                          for s in full]
            singles = [(int(a_src[s] - c * CHUNK), int(s)) for s in single]
            rows, idx_of = _build_walks(pairs_full, singles)
            assert len(rows) + 1 < 32768, len(rows)
            core_tabs.append(rows)
            for s in full:
                p, fl = idx_of[int(s)]
                idx_val[s] = p
                if fl:
                    a_src[s], b_src[s] = b_src[s], a_src[s]
                    a_dst[s], b_dst[s] = b_dst[s], a_dst[s]
            for s in single:
                p, _ = idx_of[int(s)]
                idx_val[s] = p
        subcaps.append(core_tabs)

        # eidx wrap-16, replicate 8x across partitions
        w16 = idx_val.reshape(-1, 16).T.astype(np.int16)
        eidx[m] = np.tile(w16, (8, 1))

        # S matrices: [128 part=slot&127, col = q*256 + side*128 + (dst&127)]
        sl = np.arange(total_pairs)
        qq = sl >> 7
        pr = sl & 127
        av = a_src >= 0
        n2a = dinv[np.minimum(a_src, NPAD - 1)] * dinv[a_dst] * av
        bv = b_src >= 0
        n2b = dinv[np.minimum(b_src, NPAD - 1)] * dinv[b_dst] * bv
        s_host[m][pr[av], qq[av] * 256 + (a_dst[av] & 127)] = n2a[av].astype(np.float16)
        s_host[m][pr[bv], qq[bv] * 256 + 128 + (b_dst[bv] & 127)] = n2b[bv].astype(np.float16)

    SUBCAP = max(len(t) + 1 for m in range(NCORES) for t in subcaps[m])
    SUBCAP = int(-(-SUBCAP // 16) * 16)
    assert SUBCAP <= 32767

    # self-loop diag matrices [128, TILES*128]
    s_self = np.zeros((NCORES, 128, TILES * 128), np.float16)
    jj = np.arange(PC)
    for m in range(NCORES):
        g = m * PC + jj
        val = (2.0 * dinv[g] * dinv[g]).astype(np.float16)
        s_self[m][jj & 127, (jj >> 7) * 128 + (jj & 127)] = val

    return dict(
        cap=cap, waves=waves, call_nidx=call_nidx, call_g16=call_g16,
        tile_chunks=tile_chunks, total_pairs=total_pairs, nq=nq,
        gcols16=gcols16, SUBCAP=SUBCAP, subcaps=subcaps,
        eidx=eidx, s_host=s_host, s_self=s_self,
        wave_q0=[min(tile_chunks[t][0][1] for t in wave) for wave in waves],
    )


def build_bass(plan):
    waves = plan["waves"]
    call_nidx = plan["call_nidx"]
    gcols16 = plan["gcols16"]
    SUBCAP = plan["SUBCAP"]
    nq = plan["nq"]

    nc = bacc.Bacc("TRN2", target_bir_lowering=False, debug=False)
    xt = nc.dram_tensor("xt", [NCHUNK * SUBCAP + 1, FEAT], F16, kind="ExternalInput")
    xself = nc.dram_tensor("xself", [PC, FEAT], F16, kind="ExternalInput")
    eidx_d = nc.dram_tensor("eidx", [128, gcols16], I16, kind="ExternalInput")
    s_d = nc.dram_tensor("smat", [128, nq * 256], F16, kind="ExternalInput")
    sself_d = nc.dram_tensor("sself", [128, TILES * 128], F16, kind="ExternalInput")
    w_d = nc.dram_tensor("w", [FEAT, FEAT], F16, kind="ExternalInput")
    outT = nc.dram_tensor("outT", [FEAT, PC], F16, kind="ExternalOutput")

    with tile.TileContext(nc) as tc:
        with (
            tc.tile_pool(name="meta", bufs=1) as meta,
            tc.tile_pool(name="mg", bufs=2) as mgp,
            tc.tile_pool(name="sw", bufs=2) as swp,
            tc.tile_pool(name="ms", bufs=2) as msp,
            tc.tile_pool(name="fin", bufs=4) as fin,
            tc.tile_pool(name="aggps", bufs=6, space="PSUM") as aggps,
            tc.tile_pool(name="outps", bufs=2, space="PSUM") as outps,
        ):
            sb_eidx = meta.tile([128, gcols16], I16, tag="eidx")
            nc.sync.dma_start(sb_eidx[:], eidx_d[:])
            sb_w = meta.tile([FEAT, FEAT], F16, tag="w")
            nc.sync.dma_start(sb_w[:], w_d[:])

            for g, wave in enumerate(waves):
                nsw = len(wave)
                # gathers (SWDGE): 4 calls, 512B pair descriptors
                mtiles = {}
                for c in range(NCHUNK):
                    nidx = call_nidx[g][c]
                    if nidx == 0:
                        continue
                    m2 = mgp.tile([128, nidx // 128, 256], F16, tag=f"mg{c}")
                    g16 = plan["call_g16"][g][c]
                    in_ap = xt[:, :].copy()
                    in_ap.ap = bass_rust.VecI64Pair([(FEAT, SUBCAP), (1, 256)])
                    in_ap.offset = c * SUBCAP * FEAT
                    nc.gpsimd.dma_gather(
                        m2[:, : nidx // 128, :],
                        in_ap,
                        sb_eidx[:, g16 : g16 + nidx // 16],
                        nidx, nidx, 256,
                        elem_step=FEAT,
                        single_packet=(nidx <= 1024),
                    )
                    mtiles[c] = m2

                # HWDGE loads: S block, self rows, self S
                q0 = plan["wave_q0"][g]
                nqw = sum(call_nidx[g]) // 128
                sw = swp.tile([128, nqw * 256], F16, tag="sw")
                nc.sync.dma_start(sw[:], s_d[:, q0 * 256 : (q0 + nqw) * 256])
                ms = msp.tile([128, nsw, 128], F16, tag="ms")
                r0 = wave[0] * 128
                nc.sync.dma_start(
                    ms[:], xself[r0 : r0 + nsw * 128, :].rearrange(
                        "(n p) d -> p n d", p=128))
                ssl = msp.tile([128, nsw * 128], F16, tag="ssl")
                nc.sync.dma_start(
                    ssl[:], sself_d[:, wave[0] * 128 : (wave[0] + nsw) * 128])

                for p0 in range(0, nsw, 2):
                    t0, t1 = wave[p0], wave[p0 + 1]
                    ppair = aggps.tile([128, 256], F32, tag="agg", name="agg")
                    for half, t in ((0, t0), (128, t1)):
                        first = True
                        for (c, qq, j) in plan["tile_chunks"][t]:
                            lq = qq - q0
                            for side in (0, 1):
                                nc.tensor.matmul(
                                    ppair[:, half : half + 128],
                                    mtiles[c][:, j, side * 128 : side * 128 + 128],
                                    sw[:, lq * 256 + side * 128 : lq * 256 + side * 128 + 128],
                                    start=first, stop=False,
                                    skip_group_check=True,
                                )
                                first = False
                        tig = t - wave[0]
                        nc.tensor.matmul(
                            ppair[:, half : half + 128],
                            ms[:, tig, :],
                            ssl[:, tig * 128 : tig * 128 + 128],
                            start=first, stop=True, skip_group_check=True,
                        )
                    asb = fin.tile([128, 256], F16, tag="asb")
                    nc.scalar.activation(
                        asb[:], ppair[:], mybir.ActivationFunctionType.Identity)
                    op = outps.tile([128, 256], F32, tag="op")
                    nc.tensor.matmul(op[:], sb_w[:], asb[:], skip_group_check=True)
                    osb = fin.tile([128, 256], F16, tag="osb")
                    nc.scalar.activation(
                        osb[:], op[:], mybir.ActivationFunctionType.Identity)
                    # scalar-queue HWDGE so output stores never block the
                    # sync queue's next-wave loads
                    nc.scalar.dma_start(outT[:, t0 * 128 : t0 * 128 + 256], osb[:])
    nc.compile()
    return nc


_CACHE = {}


def _get_compiled(src, dst, cnt):
    plan = build_plan(src, dst, cnt)
    key = (plan["SUBCAP"], plan["cap"].tobytes())
    if key not in _CACHE:
        _CACHE[key] = (build_bass(plan), plan)
    else:
        _CACHE[key] = (_CACHE[key][0], plan)
    return _CACHE[key]


def make_inputs(plan, x, W):
    """Per-core input maps (everything except the run itself)."""
    xf = np.asarray(x).astype(np.float16)
    SUBCAP = plan["SUBCAP"]
    in_maps = []
    for m in range(NCORES):
        xtab = np.zeros((NCHUNK * SUBCAP + 1, FEAT), np.float16)
        for c in range(NCHUNK):
            rows = np.asarray(plan["subcaps"][m][c], np.int64)
            if rows.size:
                xtab[c * SUBCAP : c * SUBCAP + rows.size] = xf[
                    np.minimum(rows + c * CHUNK, N - 1)] * (rows + c * CHUNK < N)[:, None].astype(np.float16)
        xs = np.zeros((PC, FEAT), np.float16)
        lo = m * PC
        hi = min((m + 1) * PC, N)
        xs[: hi - lo] = xf[lo:hi]
        in_maps.append({
            "xt": xtab,
            "xself": xs,
            "eidx": plan["eidx"][m],
            "smat": plan["s_host"][m],
            "sself": plan["s_self"][m],
            "w": np.asarray(W).astype(np.float16),
        })
    return in_maps


def kernel(x, edge_index, W, b):
    from concourse.bass_utils import run_bass_kernel_spmd

    x = np.asarray(x)
    edge_index = np.asarray(edge_index)
    W = np.asarray(W)
    b = np.asarray(b)
    src = edge_index[0].astype(np.int64)
    dst = edge_index[1].astype(np.int64)
    cnt = np.bincount(dst, minlength=N)

    nc, plan = _get_compiled(src, dst, cnt)
    in_maps = make_inputs(plan, x, W)
    res = run_bass_kernel_spmd(nc, in_maps, list(range(NCORES)))
    outT = np.concatenate([res.results[m]["outT"] for m in range(NCORES)], axis=1)
    return (outT[:, :N].T.astype(np.float32) + b.astype(np.float32)[None, :])
